# revision 24
# baseline (speedup 1.0000x reference)
"""Multi-head causal attention on 8 Trainium2 NeuronCores.

Sharding: core c -> batch b = c // 4, head-group g = c % 4 (4 of 16 heads).
Each core computes its 4 heads' attention and the partial W_O contraction;
the host sums the 4 head-group partials per batch (the reduce of the
tensor-parallel split).

Device-side layout is transpose-free: the host pre-transposes x and the
weights so every matmul contraction lands on the partition axis:
  qT[e,s], kT[e,s]  = W^T-chunk.T @ xT-chunk          (accum over d)
  v[m,he]           = xT-chunk.T @ WvT-chunk          (accum over d)
  sT[m,s]           = kT-slice.T @ qT-block           (scores, transposed)
  pT[m,s]           = exp(sT * 1/sqrt(e))  * mask     (ScalarE + DVE)
  zT[e,s]          += v-slice.T @ pT                  (accum over m)
  den[1,s]         += ones.T @ pT                     (softmax denominator)
  recip             = exp(-ln(den))                   (DVE)
  zn[e,s]           = zT * (ones x recip)             (PE outer-prod bcast)
  out[s,d]         += zn-slice.T @ WoT                (accum over heads)

Schedule: one continuous PE stream.  The projections for s-block j+1 and
the output projection for s-block j-1 are cut into per-matmul "filler"
thunks and woven between the attention score/PV chunks of s-block j, so
the PE never stalls on ScalarE exp or DVE latency and the HAM clock gate
stays warm.  Each head's normalize tail (recip/bcast/zn) is deferred into
the next head's chunk stream to hide the DVE round-trip.

All matmul operands fp16 (full PE rate), accumulation fp32 in PSUM.
"""

import math
from collections import deque

import numpy as np

B = 2
S = 2048
D = 2048
H = 16
E = 128
HPC = 4          # heads per core
HE = HPC * E     # 512
NC_CHUNKS = D // 128   # 16 contraction chunks of 128
NBLK = 4         # s-blocks of 512
NMT = S // 128   # 16 m-tiles of 128
SCALE = 1.0 / math.sqrt(E)
N_CORES = 8

_CACHE = {}


def _build_program():
    import concourse.bacc as bacc
    import concourse.mybir as mybir
    import concourse.tile as tile

    f16 = mybir.dt.float16
    f32 = mybir.dt.float32
    Exp = mybir.ActivationFunctionType.Exp

    nc = bacc.Bacc("TRN2", target_bir_lowering=False, debug=False,
                   num_devices=N_CORES)

    xT_d = nc.dram_tensor("xT", [D, S], f16, kind="ExternalInput")
    wq_d = nc.dram_tensor("wq", [D, HE], f16, kind="ExternalInput")
    wk_d = nc.dram_tensor("wk", [D, HE], f16, kind="ExternalInput")
    wv_d = nc.dram_tensor("wv", [D, HE], f16, kind="ExternalInput")
    woT_d = nc.dram_tensor("woT", [HE, D], f16, kind="ExternalInput")
    masks_d = nc.dram_tensor("masks", [128, 128], f16, kind="ExternalInput")
    ones_d = nc.dram_tensor("ones", [128, 129], f16, kind="ExternalInput")
    outp_d = nc.dram_tensor("outp", [S, D], f16, kind="ExternalOutput")

    with tile.TileContext(nc) as tc:
        with (
            tc.tile_pool(name="const", bufs=1) as constp,
            tc.tile_pool(name="qkv", bufs=1) as qkvp,
            tc.tile_pool(name="wpool", bufs=1) as wpool,
            tc.tile_pool(name="xring", bufs=3) as xring,
            tc.tile_pool(name="post", bufs=1) as postp,
            tc.tile_pool(name="work", bufs=2) as workp,
            tc.tile_pool(name="pt", bufs=7) as ptp,
            tc.tile_pool(name="osb", bufs=2) as osbp,
            tc.tile_pool(name="chain", bufs=3, space="PSUM") as chainp,
        ):
            ones_sb = constp.tile([128, 129], f16, tag="ones")
            woT_sb = constp.tile([128, HPC, D], f16, tag="woT")
            masks_sb = constp.tile([128, 128], f16, tag="masks")

            qT = [qkvp.tile([128, S], f16, tag=f"qT{h}", name=f"qT{h}")
                  for h in range(HPC)]
            kT = [qkvp.tile([128, S], f16, tag=f"kT{h}", name=f"kT{h}")
                  for h in range(HPC)]
            vt = [qkvp.tile([128, HE], f16, tag=f"v{m}", name=f"v{m}")
                  for m in range(NMT)]

            w_sb = {}
            for name in ("wq", "wk", "wv"):
                w_sb[name] = wpool.tile([128, NC_CHUNKS, HE], f16,
                                        tag=name, name=name + "_sb")
            wsrc = {name: dram.rearrange("(c p) n -> p c n", p=128)
                    for name, dram in
                    (("wq", wq_d), ("wk", wk_d), ("wv", wv_d))}
            src = xT_d.rearrange("(c p) s -> p c s", p=128)

            # x streamed per s-block through a 3-deep ring; block j+3
            # overwrites block j after the j projections consumed it.
            x_sb = [None] * NBLK

            def alloc_x(j):
                x_sb[j] = xring.tile([128, NC_CHUNKS, 512], f16, tag="x",
                                     name=f"x{j}")

            def load_x(j, half):
                nc.sync.dma_start(
                    x_sb[j][:, 8 * half:8 * half + 8, :],
                    src[:, 8 * half:8 * half + 8, j * 512:(j + 1) * 512])

            # ---- lead-in ---------------------------------------------
            # All input DMAs go through the sync queue in strict priority
            # order (the 16 HW engines drain it in issue order, ~330GB/s
            # aggregate); the DMA subsystem itself only starts ~8us in
            # (NEFF ucode/table loads), so nothing helps before that.
            warm_sb = constp.tile([128, 512], f16, tag="warm")
            nc.vector.memset(warm_sb[:], 0.0)
            alloc_x(0)
            # first-needed data in small pieces so no single engine's
            # serial drain gates the first matmul chain
            for c in range(4):
                nc.sync.dma_start(w_sb["wk"][:, c:c + 1, :],
                                  wsrc["wk"][:, c:c + 1, :])
                nc.sync.dma_start(x_sb[0][:, c:c + 1, :],
                                  src[:, c:c + 1, 0:512])
            # warmups bridge the first DMAs so the HAM activity window
            # is warm when the first real chain starts
            for w in range(12):
                wps = chainp.tile([1, 512], f32, tag="chain", name="wps")
                nc.tensor.matmul(wps[:], lhsT=warm_sb[:, 0:1],
                                 rhs=warm_sb[:], start=True, stop=True)
            for e6 in range(2, 8):
                nc.sync.dma_start(w_sb["wk"][:, 2 * e6:2 * e6 + 2, :],
                                  wsrc["wk"][:, 2 * e6:2 * e6 + 2, :])
                nc.sync.dma_start(x_sb[0][:, 2 * e6:2 * e6 + 2, :],
                                  src[:, 2 * e6:2 * e6 + 2, 0:512])
            for q in range(4):
                nc.sync.dma_start(w_sb["wq"][:, 4 * q:4 * q + 4, :],
                                  wsrc["wq"][:, 4 * q:4 * q + 4, :])
            for q in range(4):
                nc.sync.dma_start(w_sb["wv"][:, 4 * q:4 * q + 4, :],
                                  wsrc["wv"][:, 4 * q:4 * q + 4, :])
            nc.sync.dma_start(ones_sb[:], ones_d[:])
            for j in range(1, 3):
                alloc_x(j)
                for half in range(2):
                    nc.sync.dma_start(
                        x_sb[j][:, 8 * half:8 * half + 8, :],
                        src[:, 8 * half:8 * half + 8,
                            j * 512:(j + 1) * 512])
            wot_src = woT_d.rearrange("(c p) d -> p c d", p=128)
            for c in range(HPC):
                nc.sync.dma_start(woT_sb[:, c, :], wot_src[:, c, :])
            nc.sync.dma_start(masks_sb[:], masks_d[:])

            # ---- chain thunk machinery -------------------------------
            def proj_chain(dst_ap, lhs_of_c, rhs_of_c):
                """16-matmul accumulation chain + cast, as thunks."""
                state = {}

                def mk(c):
                    def mm():
                        if c == 0:
                            state["ps"] = chainp.tile([128, 512], f32,
                                                      tag="chain",
                                                      name="chps")
                        nc.tensor.matmul(
                            state["ps"][:], lhsT=lhs_of_c(c),
                            rhs=rhs_of_c(c),
                            start=(c == 0), stop=(c == NC_CHUNKS - 1))
                    return mm

                def cast():
                    nc.vector.tensor_copy(dst_ap, state["ps"][:])
                return [mk(c) for c in range(NC_CHUNKS)] + [cast]

            def proj_chains_for_j(j):
                thunks = []
                for h in range(HPC):
                    for dst, w in ((kT[h], w_sb["wk"]),
                                   (qT[h], w_sb["wq"])):
                        thunks += proj_chain(
                            dst[:, j * 512:(j + 1) * 512],
                            lambda c, w=w, h=h: w[:, c, h * E:(h + 1) * E],
                            lambda c, j=j: x_sb[j][:, c, :])
                for m in range(4 * j, 4 * j + 4):
                    thunks += proj_chain(
                        vt[m][:],
                        lambda c, m=m: x_sb[m // 4][:, c,
                                                    (m % 4) * 128:
                                                    (m % 4) * 128 + 128],
                        lambda c: w_sb["wv"][:, c, :])
                return thunks

            zn = [[None] * NBLK for _ in range(HPC)]

            def outproj_chain(j, st, db, row_state):
                state = {}

                def mk(h):
                    def mm():
                        if h == 0:
                            state["ps"] = chainp.tile([128, 512], f32,
                                                      tag="chain",
                                                      name="ops")
                        nc.tensor.matmul(
                            state["ps"][:],
                            lhsT=zn[h][j][:, st * 128:(st + 1) * 128],
                            rhs=woT_sb[:, h, db * 512:(db + 1) * 512],
                            start=(h == 0), stop=(h == HPC - 1))
                    return mm

                def cast_dma():
                    # one [128, 2048] row-block per st: 4KB contiguous
                    # per partition in DRAM (vs 1KB) for full DMA rate
                    if db == 0:
                        row_state["osb"] = osbp.tile([128, D], f16,
                                                     tag="osb", name="osb")
                    nc.vector.tensor_copy(
                        row_state["osb"][:, db * 512:(db + 1) * 512],
                        state["ps"][:])
                    row = j * 512 + st * 128
                    if j == NBLK - 1 and st == 3:
                        # final row-block: flush in halves so the last
                        # transfer after the last cast is only 256KB
                        if db == 1:
                            nc.sync.dma_start(
                                outp_d[row:row + 128, 0:1024],
                                row_state["osb"][:, 0:1024])
                        elif db == 3:
                            nc.sync.dma_start(
                                outp_d[row:row + 128, 1024:2048],
                                row_state["osb"][:, 1024:2048])
                    elif db == 3:
                        nc.sync.dma_start(outp_d[row:row + 128, :],
                                          row_state["osb"][:])
                return [mk(h) for h in range(HPC)] + [cast_dma]

            def outproj_chains_for_j(j):
                thunks = []
                for st in range(4):
                    row_state = {}
                    for db in range(4):
                        thunks += outproj_chain(j, st, db, row_state)
                return thunks

            fillers = deque()
            pushed = [0]
            popped = [0]

            def push_fillers(thunks):
                fillers.extend(thunks)
                pushed[0] += len(thunks)

            def emit_filler(k):
                n = 0
                while fillers and n < k:
                    fillers.popleft()()
                    popped[0] += 1
                    n += 1

            def drain_to(mark):
                while popped[0] < mark:
                    fillers.popleft()()
                    popped[0] += 1

            def drain_fillers():
                drain_to(pushed[0])

            # ---- attention head block --------------------------------
            tail_pending = [None]
            on_tail_emitted = [None]

            def head_block(j, h):
                zps = psZ.tile([128, 512], f32, tag="z")
                nchunks = 4 * j + 4
                pts = [None] * nchunks
                cols = [None] * nchunks

                def emit_score(i):
                    r = i - 4 * j
                    c0 = 128 * r if r > 0 else 0
                    cols[i] = c0
                    sps = psS.tile([128, 512], f32, tag="s", name="sps")
                    nc.tensor.matmul(
                        sps[:, c0:512],
                        lhsT=kT[h][:, i * 128:(i + 1) * 128],
                        rhs=qT[h][:, j * 512 + c0:(j + 1) * 512],
                        start=True, stop=True)
                    pt = ptp.tile([128, 512], f16, tag="pt", name="pt")
                    nc.scalar.activation(pt[:, c0:512], sps[:, c0:512],
                                         Exp, scale=SCALE)
                    if r >= 0:
                        nc.vector.tensor_mul(
                            pt[:, c0:c0 + 128], pt[:, c0:c0 + 128],
                            masks_sb[:])
                    pts[i] = pt

                colsum_state = {"t": None}

                def emit_pv(i):
                    c0 = cols[i]
                    pt = pts[i]
                    last = (i == nchunks - 1)
                    nc.tensor.matmul(
                        zps[:, c0:512],
                        lhsT=vt[i][:, h * E:(h + 1) * E],
                        rhs=pt[:, c0:512], start=(i == 0), stop=last,
                        skip_group_check=(c0 > 0))
                    # denominator numerator-sum accumulated on the DVE;
                    # the single ones-matmul happens in the tail
                    cs = colsum_state["t"]
                    if cs is None:
                        cs = workp.tile([128, 512], f16, tag="colsum",
                                        name="colsum")
                        colsum_state["t"] = cs
                        nc.vector.tensor_copy(cs[:, c0:512],
                                              pt[:, c0:512])
                    else:
                        nc.vector.tensor_add(cs[:, c0:512],
                                             cs[:, c0:512],
                                             pt[:, c0:512])
                    pts[i] = None

                def tail():
                    # [128,128] all-ones stationary weight: full-array
                    # LDWEIGHTS (no col_grp mask -> no group-switch
                    # stall) and the denominator lands pre-broadcast
                    # across all 128 partitions
                    dps = psS.tile([128, 512], f32, tag="s", name="dps")
                    nc.tensor.matmul(dps[:], lhsT=ones_sb[:, 0:128],
                                     rhs=colsum_state["t"][:],
                                     start=True, stop=True)
                    rec = workp.tile([128, 512], f32, tag="rec")
                    nc.vector.reciprocal_approx_fast(rec[:], dps[:])
                    z = postp.tile([128, 512], f16, tag=f"zn{h}_{j}",
                                   name=f"zn{h}_{j}")
                    nc.vector.tensor_mul(z[:], zps[:], rec[:])
                    zn[h][j] = z

                off = min(3, nchunks - 1)
                for i in range(nchunks):
                    emit_score(i)
                    if i == 1 and tail_pending[0] is not None:
                        tail_pending[0]()
                        tail_pending[0] = None
                        if on_tail_emitted[0] is not None:
                            on_tail_emitted[0]()
                            on_tail_emitted[0] = None
                    if i >= off:
                        emit_pv(i - off)
                    emit_filler(1)
                for i in range(nchunks - off, nchunks):
                    emit_pv(i)
                    emit_filler(1)
                if tail_pending[0] is not None:     # j==0 short blocks
                    tail_pending[0]()
                    tail_pending[0] = None
                    if on_tail_emitted[0] is not None:
                        on_tail_emitted[0]()
                        on_tail_emitted[0] = None
                tail_pending[0] = tail
                emit_filler(8)

            # ---- block-0 projections, c-interleaved ------------------
            # Four accumulation chains advance per arriving (w,x) chunk
            # pair, so the DMA-paced start keeps the PE ~4x denser than
            # sequential chains would (warms the HAM clock gate early).
            with tc.tile_pool(name="lead", bufs=4, space="PSUM") as leadp:
                def lead_pass(dst_of_s, lhs_of, rhs_of):
                    ps = [leadp.tile([128, 512], f32, tag="lead",
                                     name=f"lps{s}") for s in range(4)]
                    for c in range(NC_CHUNKS):
                        for s in range(4):
                            nc.tensor.matmul(
                                ps[s][:], lhsT=lhs_of(c, s),
                                rhs=rhs_of(c, s),
                                start=(c == 0),
                                stop=(c == NC_CHUNKS - 1))
                    for s in range(4):
                        nc.vector.tensor_copy(dst_of_s(s), ps[s][:])

                lead_pass(lambda s: kT[s][:, 0:512],
                          lambda c, s: w_sb["wk"][:, c, s * E:(s + 1) * E],
                          lambda c, s: x_sb[0][:, c, :])
                lead_pass(lambda s: qT[s][:, 0:512],
                          lambda c, s: w_sb["wq"][:, c, s * E:(s + 1) * E],
                          lambda c, s: x_sb[0][:, c, :])
                lead_pass(lambda s: vt[s][:],
                          lambda c, s: x_sb[0][:, c,
                                              s * 128:s * 128 + 128],
                          lambda c, s: w_sb["wv"][:, c, :])

            # ---- main pipeline ---------------------------------------
            attn_pools = (
                tc.tile_pool(name="psS", bufs=3, space="PSUM"),
                tc.tile_pool(name="psZ", bufs=2, space="PSUM"),
            )
            psS = attn_pools[0].__enter__()
            psZ = attn_pools[1].__enter__()

            for j in range(NBLK):
                proj_mark = None
                if j + 1 < NBLK:
                    if j + 1 == 3:
                        alloc_x(3)
                        for half in range(2):
                            load_x(3, half)
                    push_fillers(proj_chains_for_j(j + 1))
                    proj_mark = pushed[0]
                if j > 0:
                    # once block j-1's last zn is written (deferred tail
                    # fires early in this block), its out-proj becomes
                    # filler work
                    on_tail_emitted[0] = (
                        lambda j=j: push_fillers(outproj_chains_for_j(j - 1)))
                for h in range(HPC):
                    head_block(j, h)
                if proj_mark is not None:
                    # correctness invariant: block j+1 projections must
                    # be fully emitted before attention j+1 reads them
                    drain_to(proj_mark)

            tail_pending[0]()
            tail_pending[0] = None
            drain_fillers()
            for th in outproj_chains_for_j(NBLK - 1):
                th()
            for p in reversed(attn_pools):
                p.__exit__(None, None, None)

    nc.compile()
    return nc


def _get_nc():
    if "nc" not in _CACHE:
        _CACHE["nc"] = _build_program()
    return _CACHE["nc"]


def _host_inputs(x, W_Q, W_K, W_V, W_O):
    """Per-core input dicts (all fp16, pre-transposed)."""
    cc = np.arange(128)[None, :]
    mm = np.arange(128)[:, None]
    masks = (cc >= mm).astype(np.float16)
    ones = np.ones((128, 129), dtype=np.float16)

    in_maps = []
    for c in range(N_CORES):
        b, g = divmod(c, 4)
        hs = slice(HPC * g, HPC * g + HPC)
        xT = np.ascontiguousarray(x[b].T).astype(np.float16)
        wq = np.ascontiguousarray(
            W_Q[hs].transpose(2, 0, 1).reshape(D, HE)).astype(np.float16)
        wk = np.ascontiguousarray(
            W_K[hs].transpose(2, 0, 1).reshape(D, HE)).astype(np.float16)
        wv = np.ascontiguousarray(
            W_V[hs].transpose(2, 0, 1).reshape(D, HE)).astype(np.float16)
        woT = np.ascontiguousarray(
            W_O[hs].transpose(0, 2, 1).reshape(HE, D)).astype(np.float16)
        in_maps.append({"xT": xT, "wq": wq, "wk": wk, "wv": wv,
                        "woT": woT, "masks": masks, "ones": ones})
    return in_maps


def _run(in_maps, trace=False, **kw):
    from concourse.bass_utils import run_bass_kernel_spmd
    nc = _get_nc()
    return run_bass_kernel_spmd(nc, in_maps, list(range(N_CORES)),
                                trace=trace, **kw)


def kernel(x, W_Q, W_K, W_V, W_O):
    x, W_Q, W_K, W_V, W_O = (np.asarray(a, dtype=np.float32)
                             for a in (x, W_Q, W_K, W_V, W_O))
    res = _run(_host_inputs(x, W_Q, W_K, W_V, W_O))
    parts = [np.asarray(res.results[c]["outp"], dtype=np.float32)
             for c in range(N_CORES)]
    out = np.stack([parts[0] + parts[1] + parts[2] + parts[3],
                    parts[4] + parts[5] + parts[6] + parts[7]])
    return out
